# revision 32
# baseline (speedup 1.0000x reference)
"""PiNet (2-level GCN + segment softmax + bilinear pooling) on 8 Trainium2 cores.

Strategy: graph-level data parallelism (64 graphs -> 8 graphs/core). The GCN
message passing is reformulated as dense matmuls: for each graph we build the
dense normalized adjacency A = D^-1/2 (C + I) D^-1/2 (1024x1024, bf16) on the
host and stream it through the PE twice (once per GCN level, both branches
fused into 128 feature columns). Everything on device is feature-major so
bias/relu/softmax are per-partition ops; the bilinear operands are flipped to
node-major with XBAR DMA transposes.
"""

import sys

if "/opt/trn_rl_repo" not in sys.path:
    sys.path.insert(0, "/opt/trn_rl_repo")

import numpy as np
import ml_dtypes

import concourse.bacc as bacc
import concourse.tile as tile
import concourse.mybir as mybir
from concourse.bass_utils import run_bass_kernel_spmd

F32 = mybir.dt.float32
BF16 = mybir.dt.bfloat16
AF = mybir.ActivationFunctionType
ALU = mybir.AluOpType
AX = mybir.AxisListType

G = 64
NPG = 1024
N = G * NPG
E = 2097152
F_IN = 128
D0 = 64
SKIP2 = 128
NCORES = 8
GPC = G // NCORES  # graphs per core
NPC = GPC * NPG    # nodes per core

_CACHE = {}
LAST_RESULTS = None


def _emit(nc, tc, cpool, apool, wpool, fpool, spool, pmm, aps):
    (d_xt, d_adj, d_pbf, d_pf32, d_flat, d_bo) = aps

    # ---- constants into SBUF (packed: 2 DMAs on the ACT queue) ----
    pbf = cpool.tile([128, 512], BF16)
    nc.scalar.dma_start(pbf[:], d_pbf[:])
    pf32 = cpool.tile([128, 259], F32)
    nc.scalar.dma_start(pf32[:], d_pf32[:])
    xt = cpool.tile([128, NPC], BF16)
    w1cat = pbf[:, 0:128]
    wmid_h1 = pbf[:, 128:256]
    wmid_x = pbf[:, 256:384]
    w2bd = pbf[:, 384:512]
    w2s = pf32[:, 0:256]
    b1cat = pf32[:, 256:257]
    bmid = pf32[:, 257:258]
    b2cat = pf32[:, 258:259]
    ones = cpool.tile([128, 1], F32)
    nc.vector.memset(ones[:], 1.0)
    r_all = cpool.tile([128, 2 * GPC], F32)
    bo_sb = cpool.tile([16, 1], F32)

    HALF = 512

    state = {}

    def front_a(g):
        # per-graph input loads + S1 (feature-major) + cast + transpose
        xg = xt[:, g * NPG:(g + 1) * NPG]
        nc.scalar.dma_start(xg[:], d_xt[:, g * NPG:(g + 1) * NPG])
        a_sb = apool.tile([128, 8, NPG], BF16, name=f"a_sb_{g}", tag="a_sb")
        adj_r = d_adj[g].rearrange("(c p) d -> p c d", p=128)
        if g == 0:
            for c in range(8):
                nc.sync.dma_start(a_sb[:, c, :], adj_r[:, c, :])
        else:
            nc.sync.dma_start(a_sb[:], adj_r[:])

        hw1 = wpool.tile([128, 8, 128], BF16, name=f"hw1_{g}", tag="hw1")
        for h in range(2):
            ps1 = pmm.tile([128, 4, 128], F32, tag="ps", name=f"ps1_{g}_{h}")
            for c4 in range(4):
                c = 4 * h + c4
                nc.tensor.matmul(
                    ps1[:, c4, :],
                    lhsT=xt[:, (g * 8 + c) * 128:(g * 8 + c + 1) * 128],
                    rhs=w1cat, start=True, stop=True)
            nc.vector.tensor_copy(hw1[:, 4 * h:4 * (h + 1), :], ps1[:])
        state[g] = dict(xg=xg, a_sb=a_sb, hw1=hw1)

    def front_b(g):
        st = state[g]
        xg, a_sb, hw1 = st["xg"], st["a_sb"], st["hw1"]
        # ---- S2: agg1[feat, dst] = sum_s hw1[s, feat] * A[s, dst] ----
        agg1 = [pmm.tile([128, HALF], F32, tag="ps", name=f"agg1_{g}_{h}")
                for h in range(2)]
        for c in range(8):
            for h in range(2):
                nc.tensor.matmul(
                    agg1[h][:], lhsT=hw1[:, c, :],
                    rhs=a_sb[:, c, h * HALF:(h + 1) * HALF],
                    start=(c == 0), stop=(c == 7))
        h1 = wpool.tile([128, NPG], BF16, name=f"h1_{g}", tag="h1")
        for h in range(2):
            nc.scalar.activation(h1[:, h * HALF:(h + 1) * HALF], agg1[h][:],
                                 AF.Relu, bias=b1cat)

        # ---- S4: [a1L|x1L] = blockdiag linears over [h1 | x] ----
        h1L = wpool.tile([128, NPG], BF16, name=f"h1L_{g}", tag="h1L")
        acf = wpool.tile([128, NPG], F32, name=f"acf_{g}", tag="acf")
        for h in range(2):
            sl = slice(h * HALF, (h + 1) * HALF)
            ps4 = pmm.tile([128, HALF], F32, tag="ps", name=f"ps4_{g}_{h}")
            nc.tensor.matmul(ps4[:], lhsT=wmid_x, rhs=xg[:, sl],
                             start=True, stop=False)
            nc.tensor.matmul(ps4[:], lhsT=wmid_h1, rhs=h1[:, sl],
                             start=False, stop=True)
            nc.vector.tensor_scalar(h1L[:, sl], ps4[:], bmid, None,
                                    op0=ALU.add)
            nc.scalar.activation(acf[0:64, sl], ps4[0:64, :], AF.Identity,
                                 bias=bmid[0:64, :])

        # ---- S5: hw2[node, feat] node-major ----
        hw2 = wpool.tile([128, 8, 128], BF16, name=f"hw2_{g}", tag="hw1")
        for h in range(2):
            ps5 = pmm.tile([128, 4, 128], F32, tag="ps", name=f"ps5_{g}_{h}")
            for c4 in range(4):
                c = 4 * h + c4
                nc.tensor.matmul(
                    ps5[:, c4, :], lhsT=h1L[:, c * 128:(c + 1) * 128],
                    rhs=w2bd, start=True, stop=True)
            nc.vector.tensor_copy(hw2[:, 4 * h:4 * (h + 1), :], ps5[:])
        st.update(h1=h1, h1L=h1L, acf=acf, hw2=hw2)

    def front_c(g):
        st = state[g]
        a_sb, h1L, acf, hw2 = st["a_sb"], st["h1L"], st["acf"], st["hw2"]
        # ---- S6: agg2 ----
        agg2 = [pmm.tile([128, HALF], F32, tag="ps", name=f"agg2_{g}_{h}")
                for h in range(2)]
        xcs = wpool.tile([128, NPG], BF16, name=f"xcs_{g}", tag="xcs")
        for c in range(8):
            for h in range(2):
                nc.tensor.matmul(
                    agg2[h][:], lhsT=hw2[:, c, :],
                    rhs=a_sb[:, c, h * HALF:(h + 1) * HALF],
                    start=(c == 0), stop=(c == 7))
        for h in range(2):
            sl = slice(h * HALF, (h + 1) * HALF)
            nc.scalar.activation(xcs[0:64, sl], agg2[h][0:64, :], AF.Relu,
                                 bias=b2cat[0:64, :])
            nc.scalar.activation(acf[64:128, sl], agg2[h][64:128, :],
                                 AF.Identity, bias=b2cat[64:128, :])

        # ---- S8: softmax over nodes, per feature (acf = [a1L | a2]) ----
        mx = spool.tile([128, 1], F32, name=f"mx_{g}", tag="mx")
        nc.vector.tensor_reduce(mx[:], acf[:], axis=AX.X, op=ALU.max, negate=True)
        e_t = wpool.tile([128, NPG], BF16, name=f"e_t_{g}", tag="e_t")
        s_t = spool.tile([128, 1], F32, name=f"s_t_{g}", tag="s_t")
        nc.scalar.activation(e_t[:], acf[:], AF.Exp, bias=mx[:, 0:1],
                             accum_out=s_t[:, 0:1])
        rs = spool.tile([128, 1], F32, name=f"rs_{g}", tag="rs")
        nc.vector.reciprocal(rs[:], s_t[:])
        asm = wpool.tile([128, NPG], BF16, name=f"asm_{g}", tag="asm")
        nc.vector.tensor_scalar(asm[:], e_t[:], rs[:, 0:1], None, op0=ALU.mult)
        st.update(xcs=xcs, asm=asm)

    def back_t(g):
        # transposes to node-major (SP queue, ahead of next A-load)
        st = state[g]
        a_nm = wpool.tile([128, 8, 128], BF16, name=f"a_nm_{g}", tag="a_nm")
        nc.sync.dma_start(a_nm[:, :, 0:64], st["asm"][0:64, :], transpose=True)
        nc.sync.dma_start(a_nm[:, :, 64:128], st["asm"][64:128, :], transpose=True)
        x_nm = wpool.tile([128, 8, 128], BF16, name=f"x_nm_{g}", tag="x_nm")
        nc.sync.dma_start(x_nm[:, :, 0:64], st["xcs"][0:64, :], transpose=True)
        nc.sync.dma_start(x_nm[:, :, 64:128], st["h1L"][64:128, :], transpose=True)
        st.update(a_nm=a_nm, x_nm=x_nm)

    def bil(g, cs):
        # bilinear chunk matmuls (accumulated into psb)
        st = state[g]
        if "psb" not in st:
            st["psb"] = pmm.tile([128, 128], F32, tag="ps", name=f"psb_{g}")
        for c in cs:
            nc.tensor.matmul(st["psb"][:], lhsT=st["a_nm"][:, c, :],
                             rhs=st["x_nm"][:, c, :],
                             start=(c == 0), stop=(c == 7))

    def back_out(g):
        st = state[g]
        flat_sb = fpool.tile([128, 128], F32, name=f"flat_sb_{g}", tag="flat")
        nc.vector.tensor_copy(flat_sb[:], st["psb"][:])
        flats.append(flat_sb)
        # storage rows [a1L|a2] -> actual rows [a2|a1L]
        nc.sync.dma_start(
            d_flat[g, 0:64 * 128].rearrange("(p f) -> p f", f=128),
            flat_sb[64:128, :])
        nc.sync.dma_start(
            d_flat[g, 64 * 128:128 * 128].rearrange("(p f) -> p f", f=128),
            flat_sb[0:64, :])
        del state[g]

    def batchout(g):
        for cl in range(2):
            t = wpool.tile([128, 128], F32, name=f"t_{g}_{cl}", tag="t")
            nc.gpsimd.tensor_tensor(t[:], flats[g][:],
                                    w2s[:, cl * 128:(cl + 1) * 128], op=ALU.mult)
            nc.vector.tensor_reduce(r_all[:, 2 * g + cl:2 * g + cl + 1], t[:],
                                    axis=AX.X, op=ALU.add)

    # software pipeline, 1-graph skew: back(g-1) emitted after front(g)
    flats = []
    for it in range(GPC + 1):
        if it < GPC:
            front_a(it)
            front_b(it)
            front_c(it)
        if it == GPC:
            for gg in range(GPC - 1):
                batchout(gg)
        if it >= 1:
            back_t(it - 1)
            bil(it - 1, range(8))
            back_out(it - 1)
    batchout(GPC - 1)
    bo_ps = pmm.tile([16, 1], F32, tag="ps", name="bo_ps")
    nc.tensor.matmul(bo_ps[:], lhsT=r_all[:], rhs=ones[:], start=True, stop=True)
    nc.vector.tensor_copy(bo_sb[:], bo_ps[:])

    nc.sync.dma_start(d_bo.rearrange("g (c o) -> (g c) o", o=1), bo_sb[:])


def build_program():
    if "nc" in _CACHE:
        return _CACHE["nc"]
    nc = bacc.Bacc(
        "TRN2",
        target_bir_lowering=False,
        debug=False,
        enable_asserts=False,
        num_devices=NCORES,
    )
    d_xt = nc.dram_tensor("xt", [128, NPC], BF16, kind="ExternalInput").ap()
    d_adj = nc.dram_tensor("adj", [GPC, NPG, NPG], BF16, kind="ExternalInput").ap()
    d_pbf = nc.dram_tensor("pbf", [128, 512], BF16, kind="ExternalInput").ap()
    d_pf32 = nc.dram_tensor("pf32", [128, 259], F32, kind="ExternalInput").ap()
    d_flat = nc.dram_tensor("flat", [GPC, SKIP2 * SKIP2], F32, kind="ExternalOutput").ap()
    d_bo = nc.dram_tensor("bo", [GPC, 2], F32, kind="ExternalOutput").ap()
    aps = (d_xt, d_adj, d_pbf, d_pf32, d_flat, d_bo)

    with tile.TileContext(nc) as tc:
        with (
            tc.tile_pool(name="const", bufs=1) as cpool,
            tc.tile_pool(name="adj", bufs=3) as apool,
            tc.tile_pool(name="work", bufs=3) as wpool,
            tc.tile_pool(name="flatp", bufs=GPC) as fpool,
            tc.tile_pool(name="small", bufs=4) as spool,
            tc.tile_pool(name="pmm", bufs=8, space="PSUM") as pmm,
        ):
            _emit(nc, tc, cpool, apool, wpool, fpool, spool, pmm, aps)

    nc.compile()
    _CACHE["nc"] = nc
    return nc


def _host_prep(x, edge_index, batch, Wa1, ba1, Wx1, bx1, Wla, bla, Wlx, blx,
               Wa2, ba2, Wx2, bx2, W2, b2):
    bf16 = ml_dtypes.bfloat16
    src = np.asarray(edge_index[0], dtype=np.int64)
    dst = np.asarray(edge_index[1], dtype=np.int64)
    x = np.asarray(x, dtype=np.float32)

    deg = np.bincount(dst, minlength=N).astype(np.float32) + 1.0
    dinv = 1.0 / np.sqrt(deg)

    e_per = E // G
    gids = np.arange(G, dtype=np.int64)
    src_r = src.reshape(G, e_per)
    dst_r = dst.reshape(G, e_per)
    if not ((src_r >> 10) == gids[:, None]).all() or not (
            (dst_r >> 10) == gids[:, None]).all():
        # generic fallback: group edges by graph of src
        order = np.argsort(src >> 10, kind="stable")
        s_s, d_s = src[order], dst[order]
        counts = np.bincount(s_s >> 10, minlength=G)
        offs = np.concatenate([[0], np.cumsum(counts)])
        src_r = [s_s[offs[i]:offs[i + 1]] for i in range(G)]
        dst_r = [d_s[offs[i]:offs[i + 1]] for i in range(G)]

    A_all = np.empty((G, NPG, NPG), dtype=bf16)
    diag = np.arange(NPG)
    for g in range(G):
        flat = ((src_r[g] & 1023) << 10) | (dst_r[g] & 1023)
        cnt = np.bincount(flat, minlength=NPG * NPG).astype(np.float32)
        cnt = cnt.reshape(NPG, NPG)
        cnt[diag, diag] += 1.0
        dg = dinv[g * NPG:(g + 1) * NPG]
        A_all[g] = (cnt * dg[:, None]) * dg[None, :]

    xt = np.ascontiguousarray(x.T).astype(bf16)  # [128, N]

    w1cat = np.concatenate([Wa1, Wx1], axis=1).astype(bf16)          # [128,128]
    b1cat = np.concatenate([ba1, bx1]).reshape(128, 1).astype(np.float32)
    # mid-layer: out cols [a1L 0:64 | x1L 64:128]; k rows = [a1 0:64 | x1 64:128]
    wmid_h1 = np.zeros((128, 128), dtype=np.float32)
    wmid_h1[0:64, 0:64] = Wla[0:64, :]
    wmid_h1[64:128, 64:128] = Wlx[0:64, :]
    wmid_h1 = wmid_h1.astype(bf16)
    wmid_x = np.concatenate([Wla[64:192, :], Wlx[64:192, :]], axis=1).astype(bf16)
    bmid = np.concatenate([bla, blx]).reshape(128, 1).astype(np.float32)
    w2bd = np.zeros((128, 128), dtype=np.float32)
    w2bd[0:64, 64:128] = Wa2   # a1L rows -> a-branch cols 64:128
    w2bd[64:128, 0:64] = Wx2   # x1L rows -> x-branch cols 0:64
    w2bd = w2bd.astype(bf16)
    b2cat = np.concatenate([bx2, ba2]).reshape(128, 1).astype(np.float32)
    # W2 permuted to storage order: fs0:64=a1L(actual f 64:128), fs64:128=a2(actual 0:64)
    Wr = np.asarray(W2, dtype=np.float32).reshape(128, 128, 2)
    Wr = np.concatenate([Wr[64:128], Wr[0:64]], axis=0)  # [fs, hs, c]
    w2s = np.concatenate([Wr[:, :, 0], Wr[:, :, 1]], axis=1)  # [128, 256]
    w2s = np.ascontiguousarray(w2s).astype(np.float32)

    pbf = np.concatenate([w1cat, wmid_h1, wmid_x, w2bd], axis=1)  # [128, 512]
    pf32 = np.concatenate([w2s, b1cat, bmid, b2cat], axis=1)       # [128, 259]
    shared = dict(pbf=np.ascontiguousarray(pbf),
                  pf32=np.ascontiguousarray(pf32.astype(np.float32)))
    in_maps = []
    for c in range(NCORES):
        m = dict(shared)
        m["xt"] = np.ascontiguousarray(xt[:, c * NPC:(c + 1) * NPC])
        m["adj"] = np.ascontiguousarray(A_all[c * GPC:(c + 1) * GPC])
        in_maps.append(m)
    return in_maps


def kernel(**inputs):
    global LAST_RESULTS
    import os
    # no NTFF profiling hook in this environment; keep the plain exec path
    os.environ.setdefault("BASS_NEVER_TRACE", "1")
    nc = build_program()
    in_maps = _host_prep(**{k: np.asarray(v) for k, v in inputs.items()})
    res = run_bass_kernel_spmd(nc, in_maps, core_ids=list(range(NCORES)))
    LAST_RESULTS = res
    flat = np.concatenate([res.results[c]["flat"] for c in range(NCORES)], axis=0)
    bo = np.concatenate([res.results[c]["bo"] for c in range(NCORES)], axis=0)
    b2 = np.asarray(inputs["b2"], dtype=np.float32)
    batch_out = (bo + b2[None, :]).astype(np.float32)
    return (batch_out, flat.astype(np.float32))


# revision 34
# speedup vs baseline: 1.0365x; 1.0365x over previous
"""PiNet (2-level GCN + segment softmax + bilinear pooling) on 8 Trainium2 cores.

Strategy: graph-level data parallelism (64 graphs -> 8 graphs/core). The GCN
message passing is reformulated as dense matmuls: for each graph we build the
dense normalized adjacency A = D^-1/2 (C + I) D^-1/2 (1024x1024, bf16) on the
host and stream it through the PE twice (once per GCN level, both branches
fused into 128 feature columns). Everything on device is feature-major so
bias/relu/softmax are per-partition ops; the bilinear operands are flipped to
node-major with XBAR DMA transposes.
"""

import sys

if "/opt/trn_rl_repo" not in sys.path:
    sys.path.insert(0, "/opt/trn_rl_repo")

import numpy as np
import ml_dtypes

import concourse.bacc as bacc
import concourse.tile as tile
import concourse.mybir as mybir
from concourse.bass_utils import run_bass_kernel_spmd

F32 = mybir.dt.float32
BF16 = mybir.dt.bfloat16
AF = mybir.ActivationFunctionType
ALU = mybir.AluOpType
AX = mybir.AxisListType

G = 64
NPG = 1024
N = G * NPG
E = 2097152
F_IN = 128
D0 = 64
SKIP2 = 128
NCORES = 8
GPC = G // NCORES  # graphs per core
NPC = GPC * NPG    # nodes per core

_CACHE = {}
LAST_RESULTS = None


def _emit(nc, tc, cpool, apool, wpool, fpool, spool, pmm, aps):
    (d_xt, d_adj, d_pbf, d_pf32, d_flat, d_bo) = aps

    # ---- constants into SBUF (packed: 2 DMAs on the ACT queue) ----
    pbf = cpool.tile([128, 512], BF16)
    nc.scalar.dma_start(pbf[:], d_pbf[:])
    pf32 = cpool.tile([128, 259], F32)
    nc.scalar.dma_start(pf32[:], d_pf32[:])
    xt = cpool.tile([128, NPC], BF16)
    w1cat = pbf[:, 0:128]
    wmid_h1 = pbf[:, 128:256]
    wmid_x = pbf[:, 256:384]
    w2bd = pbf[:, 384:512]
    w2s = pf32[:, 0:256]
    b1cat = pf32[:, 256:257]
    bmid = pf32[:, 257:258]
    b2cat = pf32[:, 258:259]
    ones = cpool.tile([128, 1], F32)
    nc.vector.memset(ones[:], 1.0)
    r_all = cpool.tile([128, 2 * GPC], F32)
    bo_sb = cpool.tile([16, 1], F32)

    HALF = 512

    state = {}

    def front_a(g):
        # per-graph input loads + S1 (feature-major) + cast + transpose
        xg = xt[:, g * NPG:(g + 1) * NPG]
        nc.scalar.dma_start(xg[:], d_xt[:, g * NPG:(g + 1) * NPG])
        a_sb = apool.tile([128, 8, NPG], BF16, name=f"a_sb_{g}", tag="a_sb")
        adj_r = d_adj[g].rearrange("(c p) d -> p c d", p=128)
        if g == 0:
            for c in range(8):
                nc.sync.dma_start(a_sb[:, c, :], adj_r[:, c, :])
        else:
            nc.sync.dma_start(a_sb[:], adj_r[:])

        hw1 = wpool.tile([128, 8, 128], BF16, name=f"hw1_{g}", tag="hw1")
        for h in range(2):
            ps1 = pmm.tile([128, 4, 128], F32, tag="ps", name=f"ps1_{g}_{h}")
            for c4 in range(4):
                c = 4 * h + c4
                nc.tensor.matmul(
                    ps1[:, c4, :],
                    lhsT=xt[:, (g * 8 + c) * 128:(g * 8 + c + 1) * 128],
                    rhs=w1cat, start=True, stop=True)
            nc.vector.tensor_copy(hw1[:, 4 * h:4 * (h + 1), :], ps1[:])
        state[g] = dict(xg=xg, a_sb=a_sb, hw1=hw1)

    def front_b(g):
        st = state[g]
        xg, a_sb, hw1 = st["xg"], st["a_sb"], st["hw1"]
        # ---- S2: agg1[feat, dst] = sum_s hw1[s, feat] * A[s, dst] ----
        agg1 = [pmm.tile([128, HALF], F32, tag="ps", name=f"agg1_{g}_{h}")
                for h in range(2)]
        for c in range(8):
            for h in range(2):
                nc.tensor.matmul(
                    agg1[h][:], lhsT=hw1[:, c, :],
                    rhs=a_sb[:, c, h * HALF:(h + 1) * HALF],
                    start=(c == 0), stop=(c == 7))
        h1 = wpool.tile([128, NPG], BF16, name=f"h1_{g}", tag="h1")
        for h in range(2):
            nc.scalar.activation(h1[:, h * HALF:(h + 1) * HALF], agg1[h][:],
                                 AF.Relu, bias=b1cat)

        # ---- S4: [a1L|x1L] = blockdiag linears over [h1 | x] ----
        h1L = wpool.tile([128, NPG], BF16, name=f"h1L_{g}", tag="h1L")
        acf = wpool.tile([128, NPG], F32, name=f"acf_{g}", tag="acf")
        for h in range(2):
            sl = slice(h * HALF, (h + 1) * HALF)
            ps4 = pmm.tile([128, HALF], F32, tag="ps", name=f"ps4_{g}_{h}")
            nc.tensor.matmul(ps4[:], lhsT=wmid_x, rhs=xg[:, sl],
                             start=True, stop=False)
            nc.tensor.matmul(ps4[:], lhsT=wmid_h1, rhs=h1[:, sl],
                             start=False, stop=True)
            nc.vector.tensor_scalar(h1L[:, sl], ps4[:], bmid, None,
                                    op0=ALU.add)
            nc.scalar.activation(acf[0:64, sl], ps4[0:64, :], AF.Identity,
                                 bias=bmid[0:64, :])

        # ---- S5: hw2[node, feat] node-major ----
        hw2 = wpool.tile([128, 8, 128], BF16, name=f"hw2_{g}", tag="hw1")
        for h in range(2):
            ps5 = pmm.tile([128, 4, 128], F32, tag="ps", name=f"ps5_{g}_{h}")
            for c4 in range(4):
                c = 4 * h + c4
                nc.tensor.matmul(
                    ps5[:, c4, :], lhsT=h1L[:, c * 128:(c + 1) * 128],
                    rhs=w2bd, start=True, stop=True)
            nc.vector.tensor_copy(hw2[:, 4 * h:4 * (h + 1), :], ps5[:])
        st.update(h1=h1, h1L=h1L, acf=acf, hw2=hw2)

    def front_c(g):
        st = state[g]
        a_sb, h1L, acf, hw2 = st["a_sb"], st["h1L"], st["acf"], st["hw2"]
        # ---- S6: agg2 ----
        agg2 = [pmm.tile([128, HALF], F32, tag="ps", name=f"agg2_{g}_{h}")
                for h in range(2)]
        xcs = wpool.tile([128, NPG], BF16, name=f"xcs_{g}", tag="xcs")
        for c in range(8):
            for h in range(2):
                nc.tensor.matmul(
                    agg2[h][:], lhsT=hw2[:, c, :],
                    rhs=a_sb[:, c, h * HALF:(h + 1) * HALF],
                    start=(c == 0), stop=(c == 7))
        for h in range(2):
            sl = slice(h * HALF, (h + 1) * HALF)
            nc.scalar.activation(xcs[0:64, sl], agg2[h][0:64, :], AF.Relu,
                                 bias=b2cat[0:64, :])
            nc.scalar.activation(acf[64:128, sl], agg2[h][64:128, :],
                                 AF.Identity, bias=b2cat[64:128, :])

        # ---- S8: softmax over nodes, per feature (acf = [a1L | a2]) ----
        mx = spool.tile([128, 1], F32, name=f"mx_{g}", tag="mx")
        nc.vector.tensor_reduce(mx[:], acf[:], axis=AX.X, op=ALU.max, negate=True)
        e_t = wpool.tile([128, NPG], BF16, name=f"e_t_{g}", tag="e_t")
        s_t = spool.tile([128, 1], F32, name=f"s_t_{g}", tag="s_t")
        nc.scalar.activation(e_t[:], acf[:], AF.Exp, bias=mx[:, 0:1],
                             accum_out=s_t[:, 0:1])
        rs = spool.tile([128, 1], F32, name=f"rs_{g}", tag="rs")
        nc.vector.reciprocal(rs[:], s_t[:])
        st.update(xcs=xcs, asm=e_t, rs=rs)

    def back_t(g):
        # transposes to node-major (SP queue, ahead of next A-load)
        st = state[g]
        a_nm = wpool.tile([128, 8, 128], BF16, name=f"a_nm_{g}", tag="a_nm")
        nc.sync.dma_start(a_nm[:, :, 0:64], st["asm"][0:64, :], transpose=True)
        nc.sync.dma_start(a_nm[:, :, 64:128], st["asm"][64:128, :], transpose=True)
        x_nm = wpool.tile([128, 8, 128], BF16, name=f"x_nm_{g}", tag="x_nm")
        nc.sync.dma_start(x_nm[:, :, 0:64], st["xcs"][0:64, :], transpose=True)
        nc.sync.dma_start(x_nm[:, :, 64:128], st["h1L"][64:128, :], transpose=True)
        st.update(a_nm=a_nm, x_nm=x_nm)

    def bil(g, cs):
        # bilinear chunk matmuls (accumulated into psb)
        st = state[g]
        if "psb" not in st:
            st["psb"] = pmm.tile([128, 128], F32, tag="ps", name=f"psb_{g}")
        for c in cs:
            nc.tensor.matmul(st["psb"][:], lhsT=st["a_nm"][:, c, :],
                             rhs=st["x_nm"][:, c, :],
                             start=(c == 0), stop=(c == 7))

    def back_out(g):
        st = state[g]
        # fold the softmax 1/s normalization into the bilinear output
        flat_sb = fpool.tile([128, 128], F32, name=f"flat_sb_{g}", tag="flat")
        nc.vector.tensor_scalar(flat_sb[:], st["psb"][:], st["rs"][:, 0:1],
                                None, op0=ALU.mult)
        flats.append(flat_sb)
        # storage rows [a1L|a2] -> actual rows [a2|a1L]
        nc.sync.dma_start(
            d_flat[g, 0:64 * 128].rearrange("(p f) -> p f", f=128),
            flat_sb[64:128, :])
        nc.sync.dma_start(
            d_flat[g, 64 * 128:128 * 128].rearrange("(p f) -> p f", f=128),
            flat_sb[0:64, :])
        del state[g]

    def batchout(g):
        for cl in range(2):
            t = wpool.tile([128, 128], F32, name=f"t_{g}_{cl}", tag="t")
            nc.gpsimd.tensor_tensor(t[:], flats[g][:],
                                    w2s[:, cl * 128:(cl + 1) * 128], op=ALU.mult)
            nc.vector.tensor_reduce(r_all[:, 2 * g + cl:2 * g + cl + 1], t[:],
                                    axis=AX.X, op=ALU.add)

    # software pipeline, 1-graph skew: back(g-1) emitted after front(g)
    flats = []
    for it in range(GPC + 1):
        if it < GPC:
            front_a(it)
            front_b(it)
            front_c(it)
        if it == GPC:
            for gg in range(GPC - 1):
                batchout(gg)
        if it >= 1:
            back_t(it - 1)
            bil(it - 1, range(8))
            back_out(it - 1)
    batchout(GPC - 1)
    bo_ps = pmm.tile([16, 1], F32, tag="ps", name="bo_ps")
    nc.tensor.matmul(bo_ps[:], lhsT=r_all[:], rhs=ones[:], start=True, stop=True)
    nc.vector.tensor_copy(bo_sb[:], bo_ps[:])

    nc.sync.dma_start(d_bo.rearrange("g (c o) -> (g c) o", o=1), bo_sb[:])


def build_program():
    if "nc" in _CACHE:
        return _CACHE["nc"]
    nc = bacc.Bacc(
        "TRN2",
        target_bir_lowering=False,
        debug=False,
        enable_asserts=False,
        num_devices=NCORES,
    )
    d_xt = nc.dram_tensor("xt", [128, NPC], BF16, kind="ExternalInput").ap()
    d_adj = nc.dram_tensor("adj", [GPC, NPG, NPG], BF16, kind="ExternalInput").ap()
    d_pbf = nc.dram_tensor("pbf", [128, 512], BF16, kind="ExternalInput").ap()
    d_pf32 = nc.dram_tensor("pf32", [128, 259], F32, kind="ExternalInput").ap()
    d_flat = nc.dram_tensor("flat", [GPC, SKIP2 * SKIP2], F32, kind="ExternalOutput").ap()
    d_bo = nc.dram_tensor("bo", [GPC, 2], F32, kind="ExternalOutput").ap()
    aps = (d_xt, d_adj, d_pbf, d_pf32, d_flat, d_bo)

    with tile.TileContext(nc) as tc:
        with (
            tc.tile_pool(name="const", bufs=1) as cpool,
            tc.tile_pool(name="adj", bufs=2) as apool,
            tc.tile_pool(name="work", bufs=3) as wpool,
            tc.tile_pool(name="flatp", bufs=GPC) as fpool,
            tc.tile_pool(name="small", bufs=4) as spool,
            tc.tile_pool(name="pmm", bufs=8, space="PSUM") as pmm,
        ):
            _emit(nc, tc, cpool, apool, wpool, fpool, spool, pmm, aps)

    nc.compile()
    _CACHE["nc"] = nc
    return nc


def _host_prep(x, edge_index, batch, Wa1, ba1, Wx1, bx1, Wla, bla, Wlx, blx,
               Wa2, ba2, Wx2, bx2, W2, b2):
    bf16 = ml_dtypes.bfloat16
    src = np.asarray(edge_index[0], dtype=np.int64)
    dst = np.asarray(edge_index[1], dtype=np.int64)
    x = np.asarray(x, dtype=np.float32)

    deg = np.bincount(dst, minlength=N).astype(np.float32) + 1.0
    dinv = 1.0 / np.sqrt(deg)

    e_per = E // G
    gids = np.arange(G, dtype=np.int64)
    src_r = src.reshape(G, e_per)
    dst_r = dst.reshape(G, e_per)
    if not ((src_r >> 10) == gids[:, None]).all() or not (
            (dst_r >> 10) == gids[:, None]).all():
        # generic fallback: group edges by graph of src
        order = np.argsort(src >> 10, kind="stable")
        s_s, d_s = src[order], dst[order]
        counts = np.bincount(s_s >> 10, minlength=G)
        offs = np.concatenate([[0], np.cumsum(counts)])
        src_r = [s_s[offs[i]:offs[i + 1]] for i in range(G)]
        dst_r = [d_s[offs[i]:offs[i + 1]] for i in range(G)]

    A_all = np.empty((G, NPG, NPG), dtype=bf16)
    diag = np.arange(NPG)
    for g in range(G):
        flat = ((src_r[g] & 1023) << 10) | (dst_r[g] & 1023)
        cnt = np.bincount(flat, minlength=NPG * NPG).astype(np.float32)
        cnt = cnt.reshape(NPG, NPG)
        cnt[diag, diag] += 1.0
        dg = dinv[g * NPG:(g + 1) * NPG]
        A_all[g] = (cnt * dg[:, None]) * dg[None, :]

    xt = np.ascontiguousarray(x.T).astype(bf16)  # [128, N]

    w1cat = np.concatenate([Wa1, Wx1], axis=1).astype(bf16)          # [128,128]
    b1cat = np.concatenate([ba1, bx1]).reshape(128, 1).astype(np.float32)
    # mid-layer: out cols [a1L 0:64 | x1L 64:128]; k rows = [a1 0:64 | x1 64:128]
    wmid_h1 = np.zeros((128, 128), dtype=np.float32)
    wmid_h1[0:64, 0:64] = Wla[0:64, :]
    wmid_h1[64:128, 64:128] = Wlx[0:64, :]
    wmid_h1 = wmid_h1.astype(bf16)
    wmid_x = np.concatenate([Wla[64:192, :], Wlx[64:192, :]], axis=1).astype(bf16)
    bmid = np.concatenate([bla, blx]).reshape(128, 1).astype(np.float32)
    w2bd = np.zeros((128, 128), dtype=np.float32)
    w2bd[0:64, 64:128] = Wa2   # a1L rows -> a-branch cols 64:128
    w2bd[64:128, 0:64] = Wx2   # x1L rows -> x-branch cols 0:64
    w2bd = w2bd.astype(bf16)
    b2cat = np.concatenate([bx2, ba2]).reshape(128, 1).astype(np.float32)
    # W2 permuted to storage order: fs0:64=a1L(actual f 64:128), fs64:128=a2(actual 0:64)
    Wr = np.asarray(W2, dtype=np.float32).reshape(128, 128, 2)
    Wr = np.concatenate([Wr[64:128], Wr[0:64]], axis=0)  # [fs, hs, c]
    w2s = np.concatenate([Wr[:, :, 0], Wr[:, :, 1]], axis=1)  # [128, 256]
    w2s = np.ascontiguousarray(w2s).astype(np.float32)

    pbf = np.concatenate([w1cat, wmid_h1, wmid_x, w2bd], axis=1)  # [128, 512]
    pf32 = np.concatenate([w2s, b1cat, bmid, b2cat], axis=1)       # [128, 259]
    shared = dict(pbf=np.ascontiguousarray(pbf),
                  pf32=np.ascontiguousarray(pf32.astype(np.float32)))
    in_maps = []
    for c in range(NCORES):
        m = dict(shared)
        m["xt"] = np.ascontiguousarray(xt[:, c * NPC:(c + 1) * NPC])
        m["adj"] = np.ascontiguousarray(A_all[c * GPC:(c + 1) * GPC])
        in_maps.append(m)
    return in_maps


def kernel(**inputs):
    global LAST_RESULTS
    import os
    # no NTFF profiling hook in this environment; keep the plain exec path
    os.environ.setdefault("BASS_NEVER_TRACE", "1")
    nc = build_program()
    in_maps = _host_prep(**{k: np.asarray(v) for k, v in inputs.items()})
    res = run_bass_kernel_spmd(nc, in_maps, core_ids=list(range(NCORES)))
    LAST_RESULTS = res
    flat = np.concatenate([res.results[c]["flat"] for c in range(NCORES)], axis=0)
    bo = np.concatenate([res.results[c]["bo"] for c in range(NCORES)], axis=0)
    b2 = np.asarray(inputs["b2"], dtype=np.float32)
    batch_out = (bo + b2[None, :]).astype(np.float32)
    return (batch_out, flat.astype(np.float32))


# revision 36
# speedup vs baseline: 1.0422x; 1.0056x over previous
"""PiNet (2-level GCN + segment softmax + bilinear pooling) on 8 Trainium2 cores.

Strategy: graph-level data parallelism (64 graphs -> 8 graphs/core). The GCN
message passing is reformulated as dense matmuls: for each graph we build the
dense normalized adjacency A = D^-1/2 (C + I) D^-1/2 (1024x1024, bf16) on the
host and stream it through the PE twice (once per GCN level, both branches
fused into 128 feature columns). Everything on device is feature-major so
bias/relu/softmax are per-partition ops; the bilinear operands are flipped to
node-major with XBAR DMA transposes.
"""

import sys

if "/opt/trn_rl_repo" not in sys.path:
    sys.path.insert(0, "/opt/trn_rl_repo")

import numpy as np
import ml_dtypes

import concourse.bacc as bacc
import concourse.tile as tile
import concourse.mybir as mybir
from concourse.bass_utils import run_bass_kernel_spmd

F32 = mybir.dt.float32
BF16 = mybir.dt.bfloat16
AF = mybir.ActivationFunctionType
ALU = mybir.AluOpType
AX = mybir.AxisListType

G = 64
NPG = 1024
N = G * NPG
E = 2097152
F_IN = 128
D0 = 64
SKIP2 = 128
NCORES = 8
GPC = G // NCORES  # graphs per core
NPC = GPC * NPG    # nodes per core

_CACHE = {}
LAST_RESULTS = None


def _emit(nc, tc, cpool, apool, wpool, fpool, spool, pmm, aps):
    (d_xt, d_adj, d_pbf, d_pf32, d_flat, d_bo) = aps

    # ---- constants into SBUF (packed: 2 DMAs on the ACT queue) ----
    pbf = cpool.tile([128, 512], BF16)
    nc.scalar.dma_start(pbf[:], d_pbf[:])
    pf32 = cpool.tile([128, 259], F32)
    nc.scalar.dma_start(pf32[:], d_pf32[:])
    xt = cpool.tile([128, NPC], BF16)
    w1cat = pbf[:, 0:128]
    wmid_h1 = pbf[:, 128:256]
    wmid_x = pbf[:, 256:384]
    w2bd = pbf[:, 384:512]
    w2s = pf32[:, 0:256]
    b1cat = pf32[:, 256:257]
    bmid = pf32[:, 257:258]
    b2cat = pf32[:, 258:259]
    ones = cpool.tile([128, 1], F32)
    nc.vector.memset(ones[:], 1.0)
    r_all = cpool.tile([128, 2 * GPC], F32)
    bo_sb = cpool.tile([16, 1], F32)

    HALF = 512

    state = {}

    def front_a(g):
        # per-graph input loads + S1 (node-major chunks)
        xg = xt[:, g * NPG:(g + 1) * NPG]
        nc.scalar.dma_start(xg[:], d_xt[:, g * NPG:(g + 1) * NPG])
        a_sb = apool.tile([128, 8, NPG], BF16, name=f"a_sb_{g}", tag="a_sb")
        adj_r = d_adj[g].rearrange("(c p) d -> p c d", p=128)
        if g == 0:
            for c in range(8):
                nc.sync.dma_start(a_sb[:, c, :], adj_r[:, c, :])
        else:
            nc.sync.dma_start(a_sb[:], adj_r[:])

        hw1 = wpool.tile([128, 8, 128], BF16, name=f"hw1_{g}", tag="hw1")
        for h in range(2):
            ps1 = pmm.tile([128, 4, 128], F32, tag="ps", name=f"ps1_{g}_{h}")
            for c4 in range(4):
                c = 4 * h + c4
                nc.tensor.matmul(
                    ps1[:, c4, :],
                    lhsT=xt[:, (g * 8 + c) * 128:(g * 8 + c + 1) * 128],
                    rhs=w1cat, start=True, stop=True)
            nc.vector.tensor_copy(hw1[:, 4 * h:4 * (h + 1), :], ps1[:])
        state[g] = dict(xg=xg, a_sb=a_sb, hw1=hw1)

    def front_b(g):
        st = state[g]
        xg, a_sb, hw1 = st["xg"], st["a_sb"], st["hw1"]
        # ---- S2: agg1[feat, dst] = sum_s hw1[s, feat] * A[s, dst] ----
        agg1 = [pmm.tile([128, HALF], F32, tag="ps", name=f"agg1_{g}_{h}")
                for h in range(2)]
        for c in range(8):
            for h in range(2):
                nc.tensor.matmul(
                    agg1[h][:], lhsT=hw1[:, c, :],
                    rhs=a_sb[:, c, h * HALF:(h + 1) * HALF],
                    start=(c == 0), stop=(c == 7))
        h1 = wpool.tile([128, NPG], BF16, name=f"h1_{g}", tag="h1")
        for h in range(2):
            nc.scalar.activation(h1[:, h * HALF:(h + 1) * HALF], agg1[h][:],
                                 AF.Relu, bias=b1cat)

        # ---- S4: [a1L|x1L] = blockdiag linears over [h1 | x] ----
        h1L = wpool.tile([128, NPG], BF16, name=f"h1L_{g}", tag="h1L")
        acf = wpool.tile([128, NPG], F32, name=f"acf_{g}", tag="acf")
        for h in range(2):
            sl = slice(h * HALF, (h + 1) * HALF)
            ps4 = pmm.tile([128, HALF], F32, tag="ps", name=f"ps4_{g}_{h}")
            nc.tensor.matmul(ps4[:], lhsT=wmid_x, rhs=xg[:, sl],
                             start=True, stop=False)
            nc.tensor.matmul(ps4[:], lhsT=wmid_h1, rhs=h1[:, sl],
                             start=False, stop=True)
            nc.vector.tensor_scalar(h1L[:, sl], ps4[:], bmid, None,
                                    op0=ALU.add)
            nc.scalar.activation(acf[0:64, sl], ps4[0:64, :], AF.Identity,
                                 bias=bmid[0:64, :])

        # ---- S5: hw2[node, feat] node-major ----
        hw2 = wpool.tile([128, 8, 128], BF16, name=f"hw2_{g}", tag="hw1")
        for h in range(2):
            ps5 = pmm.tile([128, 4, 128], F32, tag="ps", name=f"ps5_{g}_{h}")
            for c4 in range(4):
                c = 4 * h + c4
                nc.tensor.matmul(
                    ps5[:, c4, :], lhsT=h1L[:, c * 128:(c + 1) * 128],
                    rhs=w2bd, start=True, stop=True)
            nc.vector.tensor_copy(hw2[:, 4 * h:4 * (h + 1), :], ps5[:])
        st.update(h1=h1, h1L=h1L, acf=acf, hw2=hw2)

    def front_c(g):
        st = state[g]
        a_sb, h1L, acf, hw2 = st["a_sb"], st["h1L"], st["acf"], st["hw2"]
        # ---- S6: agg2 ----
        agg2 = [pmm.tile([128, HALF], F32, tag="ps", name=f"agg2_{g}_{h}")
                for h in range(2)]
        xcs = wpool.tile([128, NPG], BF16, name=f"xcs_{g}", tag="xcs")
        for c in range(8):
            for h in range(2):
                nc.tensor.matmul(
                    agg2[h][:], lhsT=hw2[:, c, :],
                    rhs=a_sb[:, c, h * HALF:(h + 1) * HALF],
                    start=(c == 0), stop=(c == 7))
        for h in range(2):
            sl = slice(h * HALF, (h + 1) * HALF)
            nc.scalar.activation(xcs[0:64, sl], agg2[h][0:64, :], AF.Relu,
                                 bias=b2cat[0:64, :])
            nc.scalar.activation(acf[64:128, sl], agg2[h][64:128, :],
                                 AF.Identity, bias=b2cat[64:128, :])

        # ---- S8: softmax over nodes, per feature (acf = [a1L | a2]) ----
        mx = spool.tile([128, 1], F32, name=f"mx_{g}", tag="mx")
        nc.vector.tensor_reduce(mx[:], acf[:], axis=AX.X, op=ALU.max, negate=True)
        e_t = wpool.tile([128, NPG], BF16, name=f"e_t_{g}", tag="e_t")
        s_t = spool.tile([128, 1], F32, name=f"s_t_{g}", tag="s_t")
        nc.scalar.activation(e_t[:], acf[:], AF.Exp, bias=mx[:, 0:1],
                             accum_out=s_t[:, 0:1])
        rs = spool.tile([128, 1], F32, name=f"rs_{g}", tag="rs")
        nc.vector.reciprocal(rs[:], s_t[:])
        if g == GPC - 1:
            # last graph: its back stage is fully exposed; start the x-side
            # transposes as soon as xcs/h1L are ready
            x_nm = wpool.tile([128, 8, 128], BF16, name=f"x_nm_{g}", tag="x_nm")
            nc.sync.dma_start(x_nm[:, :, 0:64], xcs[0:64, :], transpose=True)
            nc.sync.dma_start(x_nm[:, :, 64:128], h1L[64:128, :], transpose=True)
            st["x_nm"] = x_nm
        st.update(xcs=xcs, asm=e_t, rs=rs)

    def back_t(g):
        # transposes to node-major (SP queue, ahead of next A-load)
        st = state[g]
        a_nm = wpool.tile([128, 8, 128], BF16, name=f"a_nm_{g}", tag="a_nm")
        nc.sync.dma_start(a_nm[:, :, 0:64], st["asm"][0:64, :], transpose=True)
        nc.sync.dma_start(a_nm[:, :, 64:128], st["asm"][64:128, :], transpose=True)
        if "x_nm" not in st:
            x_nm = wpool.tile([128, 8, 128], BF16, name=f"x_nm_{g}", tag="x_nm")
            nc.sync.dma_start(x_nm[:, :, 0:64], st["xcs"][0:64, :], transpose=True)
            nc.sync.dma_start(x_nm[:, :, 64:128], st["h1L"][64:128, :], transpose=True)
            st["x_nm"] = x_nm
        st.update(a_nm=a_nm)

    def bil(g, cs):
        # bilinear chunk matmuls (accumulated into psb)
        st = state[g]
        if "psb" not in st:
            st["psb"] = pmm.tile([128, 128], F32, tag="ps", name=f"psb_{g}")
        for c in cs:
            nc.tensor.matmul(st["psb"][:], lhsT=st["a_nm"][:, c, :],
                             rhs=st["x_nm"][:, c, :],
                             start=(c == 0), stop=(c == 7))

    def back_out(g):
        st = state[g]
        # fold the softmax 1/s normalization into the bilinear output
        flat_sb = fpool.tile([128, 128], F32, name=f"flat_sb_{g}", tag="flat")
        nc.vector.tensor_scalar(flat_sb[:], st["psb"][:], st["rs"][:, 0:1],
                                None, op0=ALU.mult)
        flats.append(flat_sb)
        # storage rows [a1L|a2] -> actual rows [a2|a1L]
        nc.sync.dma_start(
            d_flat[g, 0:64 * 128].rearrange("(p f) -> p f", f=128),
            flat_sb[64:128, :])
        nc.sync.dma_start(
            d_flat[g, 64 * 128:128 * 128].rearrange("(p f) -> p f", f=128),
            flat_sb[0:64, :])
        del state[g]

    def batchout(g):
        for cl in range(2):
            t = wpool.tile([128, 128], F32, name=f"t_{g}_{cl}", tag="t")
            nc.gpsimd.tensor_tensor(t[:], flats[g][:],
                                    w2s[:, cl * 128:(cl + 1) * 128], op=ALU.mult)
            nc.vector.tensor_reduce(r_all[:, 2 * g + cl:2 * g + cl + 1], t[:],
                                    axis=AX.X, op=ALU.add)

    # software pipeline, 1-graph skew: back(g-1) emitted after front(g)
    flats = []
    for it in range(GPC + 1):
        if it < GPC:
            front_a(it)
            front_b(it)
            front_c(it)
        if it == GPC:
            for gg in range(GPC - 1):
                batchout(gg)
        if it >= 1:
            back_t(it - 1)
            bil(it - 1, range(8))
            back_out(it - 1)
    batchout(GPC - 1)
    bo_ps = pmm.tile([16, 1], F32, tag="ps", name="bo_ps")
    nc.tensor.matmul(bo_ps[:], lhsT=r_all[:], rhs=ones[:], start=True, stop=True)
    nc.vector.tensor_copy(bo_sb[:], bo_ps[:])

    nc.sync.dma_start(d_bo.rearrange("g (c o) -> (g c) o", o=1), bo_sb[:])


def build_program():
    if "nc" in _CACHE:
        return _CACHE["nc"]
    nc = bacc.Bacc(
        "TRN2",
        target_bir_lowering=False,
        debug=False,
        enable_asserts=False,
        num_devices=NCORES,
    )
    d_xt = nc.dram_tensor("xt", [128, NPC], BF16, kind="ExternalInput").ap()
    d_adj = nc.dram_tensor("adj", [GPC, NPG, NPG], BF16, kind="ExternalInput").ap()
    d_pbf = nc.dram_tensor("pbf", [128, 512], BF16, kind="ExternalInput").ap()
    d_pf32 = nc.dram_tensor("pf32", [128, 259], F32, kind="ExternalInput").ap()
    d_flat = nc.dram_tensor("flat", [GPC, SKIP2 * SKIP2], F32, kind="ExternalOutput").ap()
    d_bo = nc.dram_tensor("bo", [GPC, 2], F32, kind="ExternalOutput").ap()
    aps = (d_xt, d_adj, d_pbf, d_pf32, d_flat, d_bo)

    with tile.TileContext(nc) as tc:
        with (
            tc.tile_pool(name="const", bufs=1) as cpool,
            tc.tile_pool(name="adj", bufs=2) as apool,
            tc.tile_pool(name="work", bufs=3) as wpool,
            tc.tile_pool(name="flatp", bufs=GPC) as fpool,
            tc.tile_pool(name="small", bufs=4) as spool,
            tc.tile_pool(name="pmm", bufs=8, space="PSUM") as pmm,
        ):
            _emit(nc, tc, cpool, apool, wpool, fpool, spool, pmm, aps)

    nc.compile()
    _CACHE["nc"] = nc
    return nc


def _host_prep(x, edge_index, batch, Wa1, ba1, Wx1, bx1, Wla, bla, Wlx, blx,
               Wa2, ba2, Wx2, bx2, W2, b2):
    bf16 = ml_dtypes.bfloat16
    src = np.asarray(edge_index[0], dtype=np.int64)
    dst = np.asarray(edge_index[1], dtype=np.int64)
    x = np.asarray(x, dtype=np.float32)

    deg = np.bincount(dst, minlength=N).astype(np.float32) + 1.0
    dinv = 1.0 / np.sqrt(deg)

    e_per = E // G
    gids = np.arange(G, dtype=np.int64)
    src_r = src.reshape(G, e_per)
    dst_r = dst.reshape(G, e_per)
    if not ((src_r >> 10) == gids[:, None]).all() or not (
            (dst_r >> 10) == gids[:, None]).all():
        # generic fallback: group edges by graph of src
        order = np.argsort(src >> 10, kind="stable")
        s_s, d_s = src[order], dst[order]
        counts = np.bincount(s_s >> 10, minlength=G)
        offs = np.concatenate([[0], np.cumsum(counts)])
        src_r = [s_s[offs[i]:offs[i + 1]] for i in range(G)]
        dst_r = [d_s[offs[i]:offs[i + 1]] for i in range(G)]

    A_all = np.empty((G, NPG, NPG), dtype=bf16)
    diag = np.arange(NPG)
    for g in range(G):
        flat = ((src_r[g] & 1023) << 10) | (dst_r[g] & 1023)
        cnt = np.bincount(flat, minlength=NPG * NPG).astype(np.float32)
        cnt = cnt.reshape(NPG, NPG)
        cnt[diag, diag] += 1.0
        dg = dinv[g * NPG:(g + 1) * NPG]
        A_all[g] = (cnt * dg[:, None]) * dg[None, :]

    xt = np.ascontiguousarray(x.T).astype(bf16)  # [128, N]

    w1cat = np.concatenate([Wa1, Wx1], axis=1).astype(bf16)          # [128,128]
    b1cat = np.concatenate([ba1, bx1]).reshape(128, 1).astype(np.float32)
    # mid-layer: out cols [a1L 0:64 | x1L 64:128]; k rows = [a1 0:64 | x1 64:128]
    wmid_h1 = np.zeros((128, 128), dtype=np.float32)
    wmid_h1[0:64, 0:64] = Wla[0:64, :]
    wmid_h1[64:128, 64:128] = Wlx[0:64, :]
    wmid_h1 = wmid_h1.astype(bf16)
    wmid_x = np.concatenate([Wla[64:192, :], Wlx[64:192, :]], axis=1).astype(bf16)
    bmid = np.concatenate([bla, blx]).reshape(128, 1).astype(np.float32)
    w2bd = np.zeros((128, 128), dtype=np.float32)
    w2bd[0:64, 64:128] = Wa2   # a1L rows -> a-branch cols 64:128
    w2bd[64:128, 0:64] = Wx2   # x1L rows -> x-branch cols 0:64
    w2bd = w2bd.astype(bf16)
    b2cat = np.concatenate([bx2, ba2]).reshape(128, 1).astype(np.float32)
    # W2 permuted to storage order: fs0:64=a1L(actual f 64:128), fs64:128=a2(actual 0:64)
    Wr = np.asarray(W2, dtype=np.float32).reshape(128, 128, 2)
    Wr = np.concatenate([Wr[64:128], Wr[0:64]], axis=0)  # [fs, hs, c]
    w2s = np.concatenate([Wr[:, :, 0], Wr[:, :, 1]], axis=1)  # [128, 256]
    w2s = np.ascontiguousarray(w2s).astype(np.float32)

    pbf = np.concatenate([w1cat, wmid_h1, wmid_x, w2bd], axis=1)  # [128, 512]
    pf32 = np.concatenate([w2s, b1cat, bmid, b2cat], axis=1)       # [128, 259]
    shared = dict(pbf=np.ascontiguousarray(pbf),
                  pf32=np.ascontiguousarray(pf32.astype(np.float32)))
    in_maps = []
    for c in range(NCORES):
        m = dict(shared)
        m["xt"] = np.ascontiguousarray(xt[:, c * NPC:(c + 1) * NPC])
        m["adj"] = np.ascontiguousarray(A_all[c * GPC:(c + 1) * GPC])
        in_maps.append(m)
    return in_maps


def kernel(**inputs):
    global LAST_RESULTS
    import os
    # no NTFF profiling hook in this environment; keep the plain exec path
    os.environ.setdefault("BASS_NEVER_TRACE", "1")
    nc = build_program()
    in_maps = _host_prep(**{k: np.asarray(v) for k, v in inputs.items()})
    res = run_bass_kernel_spmd(nc, in_maps, core_ids=list(range(NCORES)))
    LAST_RESULTS = res
    flat = np.concatenate([res.results[c]["flat"] for c in range(NCORES)], axis=0)
    bo = np.concatenate([res.results[c]["bo"] for c in range(NCORES)], axis=0)
    b2 = np.asarray(inputs["b2"], dtype=np.float32)
    batch_out = (bo + b2[None, :]).astype(np.float32)
    return (batch_out, flat.astype(np.float32))


# revision 38
# speedup vs baseline: 1.0466x; 1.0042x over previous
"""PiNet (2-level GCN + segment softmax + bilinear pooling) on 8 Trainium2 cores.

Strategy: graph-level data parallelism (64 graphs -> 8 graphs/core). The GCN
message passing is reformulated as dense matmuls: for each graph we build the
dense normalized adjacency A = D^-1/2 (C + I) D^-1/2 (1024x1024, bf16) on the
host and stream it through the PE twice (once per GCN level, both branches
fused into 128 feature columns). Everything on device is feature-major so
bias/relu/softmax are per-partition ops; the bilinear operands are flipped to
node-major with XBAR DMA transposes.
"""

import sys

if "/opt/trn_rl_repo" not in sys.path:
    sys.path.insert(0, "/opt/trn_rl_repo")

import numpy as np
import ml_dtypes

import concourse.bacc as bacc
import concourse.tile as tile
import concourse.mybir as mybir
from concourse.bass_utils import run_bass_kernel_spmd

F32 = mybir.dt.float32
BF16 = mybir.dt.bfloat16
AF = mybir.ActivationFunctionType
ALU = mybir.AluOpType
AX = mybir.AxisListType

G = 64
NPG = 1024
N = G * NPG
E = 2097152
F_IN = 128
D0 = 64
SKIP2 = 128
NCORES = 8
GPC = G // NCORES  # graphs per core
NPC = GPC * NPG    # nodes per core

_CACHE = {}
LAST_RESULTS = None


def _emit(nc, tc, cpool, apool, wpool, fpool, spool, pmm, aps):
    (d_xt, d_adj, d_pbf, d_pf32, d_flat, d_bo) = aps

    # ---- constants into SBUF (packed: 2 DMAs on the ACT queue) ----
    pbf = cpool.tile([128, 512], BF16)
    nc.scalar.dma_start(pbf[:], d_pbf[:])
    pf32 = cpool.tile([128, 259], F32)
    nc.scalar.dma_start(pf32[:], d_pf32[:])
    xt = cpool.tile([128, NPC], BF16)
    w1cat = pbf[:, 0:128]
    wmid_h1 = pbf[:, 128:256]
    wmid_x = pbf[:, 256:384]
    w2bd = pbf[:, 384:512]
    w2s = pf32[:, 0:256]
    b1cat = pf32[:, 256:257]
    bmid = pf32[:, 257:258]
    b2cat = pf32[:, 258:259]
    ones = cpool.tile([128, 1], F32)
    nc.vector.memset(ones[:], 1.0)
    r_all = cpool.tile([128, 2 * GPC], F32)
    bo_sb = cpool.tile([16, 1], F32)

    HALF = 512

    state = {}

    def front_a(g):
        # per-graph input loads + S1 (node-major chunks)
        xg = xt[:, g * NPG:(g + 1) * NPG]
        nc.scalar.dma_start(xg[:], d_xt[:, g * NPG:(g + 1) * NPG])
        a_sb = apool.tile([128, 8, NPG], BF16, name=f"a_sb_{g}", tag="a_sb")
        adj_r = d_adj[g].rearrange("(c p) d -> p c d", p=128)
        if g == 0:
            for c in range(8):
                nc.sync.dma_start(a_sb[:, c, :], adj_r[:, c, :])
        else:
            nc.sync.dma_start(a_sb[:], adj_r[:])

        hw1 = wpool.tile([128, 8, 128], BF16, name=f"hw1_{g}", tag="hw1")
        for h in range(2):
            ps1 = pmm.tile([128, 4, 128], F32, tag="ps", name=f"ps1_{g}_{h}")
            for c4 in range(4):
                c = 4 * h + c4
                nc.tensor.matmul(
                    ps1[:, c4, :],
                    lhsT=xt[:, (g * 8 + c) * 128:(g * 8 + c + 1) * 128],
                    rhs=w1cat, start=True, stop=True)
            nc.vector.tensor_copy(hw1[:, 4 * h:4 * (h + 1), :], ps1[:])
        state[g] = dict(xg=xg, a_sb=a_sb, hw1=hw1)

    def front_b(g):
        st = state[g]
        xg, a_sb, hw1 = st["xg"], st["a_sb"], st["hw1"]
        # ---- S2: agg1[feat, dst] = sum_s hw1[s, feat] * A[s, dst] ----
        agg1 = [pmm.tile([128, HALF], F32, tag="ps", name=f"agg1_{g}_{h}")
                for h in range(2)]
        for c in range(8):
            for h in range(2):
                nc.tensor.matmul(
                    agg1[h][:], lhsT=hw1[:, c, :],
                    rhs=a_sb[:, c, h * HALF:(h + 1) * HALF],
                    start=(c == 0), stop=(c == 7))
        h1 = wpool.tile([128, NPG], BF16, name=f"h1_{g}", tag="h1")
        for h in range(2):
            nc.scalar.activation(h1[:, h * HALF:(h + 1) * HALF], agg1[h][:],
                                 AF.Relu, bias=b1cat)

        # ---- S4: [a1L|x1L] = blockdiag linears over [h1 | x] ----
        h1L = wpool.tile([128, NPG], BF16, name=f"h1L_{g}", tag="h1L")
        acf = wpool.tile([128, NPG], F32, name=f"acf_{g}", tag="acf")
        for h in range(2):
            sl = slice(h * HALF, (h + 1) * HALF)
            ps4 = pmm.tile([128, HALF], F32, tag="ps", name=f"ps4_{g}_{h}")
            nc.tensor.matmul(ps4[:], lhsT=wmid_x, rhs=xg[:, sl],
                             start=True, stop=False)
            nc.tensor.matmul(ps4[:], lhsT=wmid_h1, rhs=h1[:, sl],
                             start=False, stop=True)
            nc.vector.tensor_scalar(h1L[:, sl], ps4[:], bmid, None,
                                    op0=ALU.add)
            nc.scalar.activation(acf[0:64, sl], ps4[0:64, :], AF.Identity,
                                 bias=bmid[0:64, :])

        # ---- S5: hw2[node, feat] node-major ----
        hw2 = wpool.tile([128, 8, 128], BF16, name=f"hw2_{g}", tag="hw1")
        for h in range(2):
            ps5 = pmm.tile([128, 4, 128], F32, tag="ps", name=f"ps5_{g}_{h}")
            for c4 in range(4):
                c = 4 * h + c4
                nc.tensor.matmul(
                    ps5[:, c4, :], lhsT=h1L[:, c * 128:(c + 1) * 128],
                    rhs=w2bd, start=True, stop=True)
            nc.vector.tensor_copy(hw2[:, 4 * h:4 * (h + 1), :], ps5[:])
        st.update(h1=h1, h1L=h1L, acf=acf, hw2=hw2)

    def front_c(g):
        st = state[g]
        a_sb, h1L, acf, hw2 = st["a_sb"], st["h1L"], st["acf"], st["hw2"]
        # ---- S6: agg2 ----
        agg2 = [pmm.tile([128, HALF], F32, tag="ps", name=f"agg2_{g}_{h}")
                for h in range(2)]
        xcs = wpool.tile([128, NPG], BF16, name=f"xcs_{g}", tag="xcs")
        for c in range(8):
            for h in range(2):
                nc.tensor.matmul(
                    agg2[h][:], lhsT=hw2[:, c, :],
                    rhs=a_sb[:, c, h * HALF:(h + 1) * HALF],
                    start=(c == 0), stop=(c == 7))
        for h in range(2):
            sl = slice(h * HALF, (h + 1) * HALF)
            nc.scalar.activation(xcs[0:64, sl], agg2[h][0:64, :], AF.Relu,
                                 bias=b2cat[0:64, :])
            nc.scalar.activation(acf[64:128, sl], agg2[h][64:128, :],
                                 AF.Identity, bias=b2cat[64:128, :])

        # ---- S8: softmax over nodes, per feature (acf = [a1L | a2]) ----
        mx = spool.tile([128, 1], F32, name=f"mx_{g}", tag="mx")
        nc.vector.tensor_reduce(mx[:], acf[:], axis=AX.X, op=ALU.max, negate=True)
        e_t = wpool.tile([128, NPG], BF16, name=f"e_t_{g}", tag="e_t")
        s_t = spool.tile([128, 1], F32, name=f"s_t_{g}", tag="s_t")
        nc.scalar.activation(e_t[:], acf[:], AF.Exp, bias=mx[:, 0:1],
                             accum_out=s_t[:, 0:1])
        rs = spool.tile([128, 1], F32, name=f"rs_{g}", tag="rs")
        nc.vector.reciprocal(rs[:], s_t[:])
        if g == GPC - 1:
            # last graph: its back stage is fully exposed; start the x-side
            # transposes as soon as xcs/h1L are ready
            x_nm = wpool.tile([128, 8, 128], BF16, name=f"x_nm_{g}", tag="x_nm")
            nc.sync.dma_start(x_nm[:, :, 0:64], xcs[0:64, :], transpose=True)
            nc.sync.dma_start(x_nm[:, :, 64:128], h1L[64:128, :], transpose=True)
            st["x_nm"] = x_nm
        st.update(xcs=xcs, asm=e_t, rs=rs)

    def back_t(g):
        # transposes to node-major (SP queue, ahead of next A-load)
        st = state[g]
        a_nm = wpool.tile([128, 8, 128], BF16, name=f"a_nm_{g}", tag="a_nm")
        nc.sync.dma_start(a_nm[:, :, 0:64], st["asm"][0:64, :], transpose=True)
        nc.sync.dma_start(a_nm[:, :, 64:128], st["asm"][64:128, :], transpose=True)
        if "x_nm" not in st:
            x_nm = wpool.tile([128, 8, 128], BF16, name=f"x_nm_{g}", tag="x_nm")
            nc.sync.dma_start(x_nm[:, :, 0:64], st["xcs"][0:64, :], transpose=True)
            nc.sync.dma_start(x_nm[:, :, 64:128], st["h1L"][64:128, :], transpose=True)
            st["x_nm"] = x_nm
        st.update(a_nm=a_nm)

    def bil(g, cs):
        # bilinear chunk matmuls (accumulated into psb)
        st = state[g]
        if "psb" not in st:
            st["psb"] = pmm.tile([128, 128], F32, tag="ps", name=f"psb_{g}")
        for c in cs:
            nc.tensor.matmul(st["psb"][:], lhsT=st["a_nm"][:, c, :],
                             rhs=st["x_nm"][:, c, :],
                             start=(c == 0), stop=(c == 7))

    def back_out(g):
        st = state[g]
        # fold the softmax 1/s normalization into the bilinear output
        flat_sb = fpool.tile([128, 128], F32, name=f"flat_sb_{g}", tag="flat")
        nc.vector.tensor_scalar(flat_sb[:], st["psb"][:], st["rs"][:, 0:1],
                                None, op0=ALU.mult)
        flats.append(flat_sb)
        # storage rows [a1L|a2] -> actual rows [a2|a1L]
        nc.sync.dma_start(
            d_flat[g, 0:64 * 128].rearrange("(p f) -> p f", f=128),
            flat_sb[64:128, :])
        nc.sync.dma_start(
            d_flat[g, 64 * 128:128 * 128].rearrange("(p f) -> p f", f=128),
            flat_sb[0:64, :])
        del state[g]

    def batchout(g):
        for cl in range(2):
            t = wpool.tile([128, 128], F32, name=f"t_{g}_{cl}", tag="t")
            nc.gpsimd.tensor_tensor(t[:], flats[g][:],
                                    w2s[:, cl * 128:(cl + 1) * 128], op=ALU.mult)
            nc.vector.tensor_reduce(r_all[:, 2 * g + cl:2 * g + cl + 1], t[:],
                                    axis=AX.X, op=ALU.add)

    # software pipeline, 1-graph skew: back(g-1) emitted after front(g)
    flats = []
    front_a(0)
    for it in range(GPC + 1):
        if it < GPC:
            if it + 1 < GPC:
                front_a(it + 1)
            front_b(it)
            front_c(it)
        if it == GPC:
            for gg in range(GPC - 1):
                batchout(gg)
        if it >= 1:
            back_t(it - 1)
            bil(it - 1, range(8))
            back_out(it - 1)
    batchout(GPC - 1)
    bo_ps = pmm.tile([16, 1], F32, tag="ps", name="bo_ps")
    nc.tensor.matmul(bo_ps[:], lhsT=r_all[:], rhs=ones[:], start=True, stop=True)
    nc.vector.tensor_copy(bo_sb[:], bo_ps[:])

    nc.sync.dma_start(d_bo.rearrange("g (c o) -> (g c) o", o=1), bo_sb[:])


def build_program():
    if "nc" in _CACHE:
        return _CACHE["nc"]
    nc = bacc.Bacc(
        "TRN2",
        target_bir_lowering=False,
        debug=False,
        enable_asserts=False,
        num_devices=NCORES,
    )
    d_xt = nc.dram_tensor("xt", [128, NPC], BF16, kind="ExternalInput").ap()
    d_adj = nc.dram_tensor("adj", [GPC, NPG, NPG], BF16, kind="ExternalInput").ap()
    d_pbf = nc.dram_tensor("pbf", [128, 512], BF16, kind="ExternalInput").ap()
    d_pf32 = nc.dram_tensor("pf32", [128, 259], F32, kind="ExternalInput").ap()
    d_flat = nc.dram_tensor("flat", [GPC, SKIP2 * SKIP2], F32, kind="ExternalOutput").ap()
    d_bo = nc.dram_tensor("bo", [GPC, 2], F32, kind="ExternalOutput").ap()
    aps = (d_xt, d_adj, d_pbf, d_pf32, d_flat, d_bo)

    with tile.TileContext(nc) as tc:
        with (
            tc.tile_pool(name="const", bufs=1) as cpool,
            tc.tile_pool(name="adj", bufs=2) as apool,
            tc.tile_pool(name="work", bufs=3) as wpool,
            tc.tile_pool(name="flatp", bufs=GPC) as fpool,
            tc.tile_pool(name="small", bufs=4) as spool,
            tc.tile_pool(name="pmm", bufs=8, space="PSUM") as pmm,
        ):
            _emit(nc, tc, cpool, apool, wpool, fpool, spool, pmm, aps)

    nc.compile()
    _CACHE["nc"] = nc
    return nc


def _host_prep(x, edge_index, batch, Wa1, ba1, Wx1, bx1, Wla, bla, Wlx, blx,
               Wa2, ba2, Wx2, bx2, W2, b2):
    bf16 = ml_dtypes.bfloat16
    src = np.asarray(edge_index[0], dtype=np.int64)
    dst = np.asarray(edge_index[1], dtype=np.int64)
    x = np.asarray(x, dtype=np.float32)

    deg = np.bincount(dst, minlength=N).astype(np.float32) + 1.0
    dinv = 1.0 / np.sqrt(deg)

    e_per = E // G
    gids = np.arange(G, dtype=np.int64)
    src_r = src.reshape(G, e_per)
    dst_r = dst.reshape(G, e_per)
    if not ((src_r >> 10) == gids[:, None]).all() or not (
            (dst_r >> 10) == gids[:, None]).all():
        # generic fallback: group edges by graph of src
        order = np.argsort(src >> 10, kind="stable")
        s_s, d_s = src[order], dst[order]
        counts = np.bincount(s_s >> 10, minlength=G)
        offs = np.concatenate([[0], np.cumsum(counts)])
        src_r = [s_s[offs[i]:offs[i + 1]] for i in range(G)]
        dst_r = [d_s[offs[i]:offs[i + 1]] for i in range(G)]

    A_all = np.empty((G, NPG, NPG), dtype=bf16)
    diag = np.arange(NPG)
    for g in range(G):
        flat = ((src_r[g] & 1023) << 10) | (dst_r[g] & 1023)
        cnt = np.bincount(flat, minlength=NPG * NPG).astype(np.float32)
        cnt = cnt.reshape(NPG, NPG)
        cnt[diag, diag] += 1.0
        dg = dinv[g * NPG:(g + 1) * NPG]
        A_all[g] = (cnt * dg[:, None]) * dg[None, :]

    xt = np.ascontiguousarray(x.T).astype(bf16)  # [128, N]

    w1cat = np.concatenate([Wa1, Wx1], axis=1).astype(bf16)          # [128,128]
    b1cat = np.concatenate([ba1, bx1]).reshape(128, 1).astype(np.float32)
    # mid-layer: out cols [a1L 0:64 | x1L 64:128]; k rows = [a1 0:64 | x1 64:128]
    wmid_h1 = np.zeros((128, 128), dtype=np.float32)
    wmid_h1[0:64, 0:64] = Wla[0:64, :]
    wmid_h1[64:128, 64:128] = Wlx[0:64, :]
    wmid_h1 = wmid_h1.astype(bf16)
    wmid_x = np.concatenate([Wla[64:192, :], Wlx[64:192, :]], axis=1).astype(bf16)
    bmid = np.concatenate([bla, blx]).reshape(128, 1).astype(np.float32)
    w2bd = np.zeros((128, 128), dtype=np.float32)
    w2bd[0:64, 64:128] = Wa2   # a1L rows -> a-branch cols 64:128
    w2bd[64:128, 0:64] = Wx2   # x1L rows -> x-branch cols 0:64
    w2bd = w2bd.astype(bf16)
    b2cat = np.concatenate([bx2, ba2]).reshape(128, 1).astype(np.float32)
    # W2 permuted to storage order: fs0:64=a1L(actual f 64:128), fs64:128=a2(actual 0:64)
    Wr = np.asarray(W2, dtype=np.float32).reshape(128, 128, 2)
    Wr = np.concatenate([Wr[64:128], Wr[0:64]], axis=0)  # [fs, hs, c]
    w2s = np.concatenate([Wr[:, :, 0], Wr[:, :, 1]], axis=1)  # [128, 256]
    w2s = np.ascontiguousarray(w2s).astype(np.float32)

    pbf = np.concatenate([w1cat, wmid_h1, wmid_x, w2bd], axis=1)  # [128, 512]
    pf32 = np.concatenate([w2s, b1cat, bmid, b2cat], axis=1)       # [128, 259]
    shared = dict(pbf=np.ascontiguousarray(pbf),
                  pf32=np.ascontiguousarray(pf32.astype(np.float32)))
    in_maps = []
    for c in range(NCORES):
        m = dict(shared)
        m["xt"] = np.ascontiguousarray(xt[:, c * NPC:(c + 1) * NPC])
        m["adj"] = np.ascontiguousarray(A_all[c * GPC:(c + 1) * GPC])
        in_maps.append(m)
    return in_maps


def kernel(**inputs):
    global LAST_RESULTS
    import os
    # no NTFF profiling hook in this environment; keep the plain exec path
    os.environ.setdefault("BASS_NEVER_TRACE", "1")
    nc = build_program()
    in_maps = _host_prep(**{k: np.asarray(v) for k, v in inputs.items()})
    res = run_bass_kernel_spmd(nc, in_maps, core_ids=list(range(NCORES)))
    LAST_RESULTS = res
    flat = np.concatenate([res.results[c]["flat"] for c in range(NCORES)], axis=0)
    bo = np.concatenate([res.results[c]["bo"] for c in range(NCORES)], axis=0)
    b2 = np.asarray(inputs["b2"], dtype=np.float32)
    batch_out = (bo + b2[None, :]).astype(np.float32)
    return (batch_out, flat.astype(np.float32))
